# revision 22
# baseline (speedup 1.0000x reference)
"""Trainium2 Bass kernel for nn_DecoderBlock (B=2, L=2048, E=1024, H=16, D=64, DFF=4096).

Strategy (8 NeuronCores, SPMD):
  - Attention head-parallel (2 heads/core), everything else token-parallel
    (512 own tokens/core).  Head->token reshard via one AllToAll per head
    (fp8 payload), first overlapped with second head's compute.
  - fp8e4 + DoubleRow matmuls for: attention probs x V, out-proj, GLU FFN
    (u/v/Wout).  QKV projection and QK^T stay bf16/f32r for accuracy.
  - exp on ACT with direct fp8 output; masked tiles get a DVE mask-add first.
  - rmsnorm sum-of-squares via fp8 squares + DoubleRow ones-contraction;
    partition-broadcast of the scale via a K=1 PE matmul + ACT copy
    (gpsimd.partition_broadcast is ~7.6us per [128,512] - too slow).
  - X loaded once as [128, ET, T] bf16 in 4 chunks (2KB DMA lines), phase 1
    consumes it straight from SBUF.
  - ctx chunks DMA'd straight into the AllToAll staging dram during
    attention; FFN weights prefetched during the second AllToAll.
  - Output written per e-tile in bf16 as soon as its Wout accumulation ends.
"""

import sys
import types

import numpy as np
import ml_dtypes

sys.path.insert(0, "/opt/trn_rl_repo")
sys.path.insert(0, "/opt/pypackages")

import concourse.bass as bass
import concourse.mybir as mybir
from concourse import bacc
import concourse.tile as tile
from concourse.bass_utils import run_bass_kernel_spmd

F32 = mybir.dt.float32
F32R = mybir.dt.float32r
BF16 = mybir.dt.bfloat16
FP8 = mybir.dt.float8e4
AF = mybir.ActivationFunctionType
ALU = mybir.AluOpType
DR = mybir.MatmulPerfMode.DoubleRow

B, L, E, H, D, DFF = 2, 2048, 1024, 16, 64, 4096
T = B * L            # 4096 flat tokens
NC_ = 8              # cores
TOK = T // NC_       # 512 own tokens per core
ET = E // 128        # 8 e-tiles
DT = DFF // 128      # 32 dff-tiles
QC = 512             # q-chunk width in attention
NQC = L // QC        # 4 q-chunks per sequence
SW = 64.0            # fp8 weight scale
SA = 16.0            # fp8 activation scale
SQ = 1.0             # fp8 squares scale (>1 overflows fp8: max x3^2*8 > 240)
SWA = SW * SA        # matmul output scale

_CACHE = {}


def install_ntff_hook():
    """Synthesize antenv.axon_hooks so trace=True can profile via libaxon_pjrt."""
    try:
        from antenv.axon_hooks import get_axon_ntff_profile_hook  # noqa
        return
    except ImportError:
        pass
    try:
        import antenv
        mod = types.ModuleType("antenv.axon_hooks")
        mod._hook = None
        mod.set_axon_ntff_profile_hook = lambda h: setattr(mod, "_hook", h)
        mod.get_axon_ntff_profile_hook = lambda: mod._hook
        sys.modules["antenv.axon_hooks"] = mod
        antenv.axon_hooks = mod
        if "/root/.axon_site" not in sys.path:
            sys.path.insert(0, "/root/.axon_site")
        from trn_agent_boot.trn_boot import _ntff_profile_via_ctypes
        mod.set_axon_ntff_profile_hook(
            _ntff_profile_via_ctypes("/opt/axon/libaxon_pjrt.so")
        )
    except Exception:
        pass


def build_nc():
    nc = bacc.Bacc("TRN2", target_bir_lowering=False, debug=False)

    # ---- I/O ----
    xt_bf = nc.dram_tensor("xt_bf", [8, 128, ET, T // 8], BF16, kind="ExternalInput")
    xt_own = nc.dram_tensor("xt_own", [128, ET, TOK], BF16, kind="ExternalInput")
    wqkv_d = nc.dram_tensor("wqkv_d", [128, ET, 3, 128], BF16, kind="ExternalInput")
    bqkv = nc.dram_tensor("bqkv", [128, 3], F32, kind="ExternalInput")
    wo_d = nc.dram_tensor("wo_d", [2, 128, 2, 2, ET, 128], FP8, kind="ExternalInput")
    wwv_d = nc.dram_tensor("wwv_d", [DT, 128, 2, 4, 2, 128], FP8, kind="ExternalInput")
    wout_d = nc.dram_tensor("wout_d", [128, DT // 2, 2, ET, 128], FP8, kind="ExternalInput")
    onesd = nc.dram_tensor("onesd", [128, 1], F32R, kind="ExternalInput")
    ones8d = nc.dram_tensor("ones8d", [128, 2, 16], FP8, kind="ExternalInput")
    onesrowd = nc.dram_tensor("onesrowd", [1, 128], F32R, kind="ExternalInput")
    extkd = nc.dram_tensor("extkd", [2, 4, T], BF16, kind="ExternalInput")
    extqd = nc.dram_tensor("extqd", [2, 4, T], BF16, kind="ExternalInput")
    mbsd = nc.dram_tensor("mbsd", [128, 2, QC], F32R, kind="ExternalInput")
    mbdd = nc.dram_tensor("mbdd", [128, 2, QC // 2], F32R, kind="ExternalInput")
    identd = nc.dram_tensor("identd", [128, 128], F32R, kind="ExternalInput")
    vonesd = nc.dram_tensor("vonesd", [128, 64], FP8, kind="ExternalInput")

    y = nc.dram_tensor("y", [ET, 128, TOK], BF16, kind="ExternalOutput")

    a2a_in = [nc.dram_tensor(f"a2a_in{h}", [NC_, 64, TOK], FP8) for h in range(2)]
    a2a_out = [nc.dram_tensor(f"a2a_out{h}", [NC_, 64, TOK], FP8) for h in range(2)]
    a2aw_in = nc.dram_tensor("a2aw_in", [NC_, 16], FP8)
    a2aw_out = nc.dram_tensor("a2aw_out", [NC_, 16], FP8)

    with tile.TileContext(nc) as tc:
        from contextlib import ExitStack
        estack = ExitStack()
        const = estack.enter_context(tc.tile_pool(name="const", bufs=1))
        ones_sb = const.tile([128, 1], F32R)
        nc.sync.dma_start(out=ones_sb, in_=onesd[:])
        ones8_sb = const.tile([128, 2, 16], FP8, tag="ones8")
        nc.sync.dma_start(out=ones8_sb, in_=ones8d[:])
        onesrow_sb = const.tile([1, 128], F32R, tag="onesrow")
        nc.sync.dma_start(out=onesrow_sb, in_=onesrowd[:])
        mbs_sb = const.tile([128, 2, QC], F32R, tag="mbs")
        nc.sync.dma_start(out=mbs_sb, in_=mbsd[:])
        mbd_sb = const.tile([128, 2, QC // 2], F32R, tag="mbd")
        nc.sync.dma_start(out=mbd_sb, in_=mbdd[:])
        ident_sb = const.tile([128, 128], F32R)
        nc.sync.dma_start(out=ident_sb, in_=identd[:])
        bqkv_sb = const.tile([128, 3], F32)
        nc.sync.dma_start(out=bqkv_sb, in_=bqkv[:])

        # phase-3 weight pool (opened early for pool stack order; DMAs are
        # emitted after phase 1 so they run during attention)
        p3w_stack = ExitStack()
        p3w = p3w_stack.enter_context(tc.tile_pool(name="p3w", bufs=1))

        # ---------- persistent attention state ----------
        att_stack = ExitStack()
        att_pool = att_stack.enter_context(tc.tile_pool(name="att", bufs=1))
        qhat = [att_pool.tile([68, T], BF16, name=f"qhat{h}", tag=f"qhat{h}") for h in range(2)]
        khat = [att_pool.tile([68, T], BF16, name=f"khat{h}", tag=f"khat{h}") for h in range(2)]
        # V natural layout: [128 k-tokens, jp(=T//256), hh, kt-in-pair, 80]
        # col 64 = ones (rowsum trick), cols 65..79 pad for step%16==0
        vnat = att_pool.tile([128, T // 256, 2, 2, 80], FP8, tag="vnat")
        nc.sync.dma_start(out=vnat[:, :, :, :, 64:65], in_=vonesd[:])

        # alibi extension rows (hi/lo split, precomputed host-side)
        for hh in range(2):
            nc.sync.dma_start(out=qhat[hh][64:68, :], in_=extqd[hh])
            nc.sync.dma_start(out=khat[hh][64:68, :], in_=extkd[hh])
        # tiny warm-up collective so the real A2As skip the ncfw cold start
        nc.gpsimd.collective_compute(
            "AllToAll", ALU.bypass, replica_groups=[list(range(NC_))],
            ins=[a2aw_in[:]], outs=[a2aw_out[:]],
        )

        # ================= Phase 1: rmsnorm1 + QKV projections =================
        NCH = 8                      # X load chunks
        CHT = T // NCH               # tokens per chunk
        with (
            tc.tile_pool(name="xch", bufs=2) as xch,
            tc.tile_pool(name="p1", bufs=2) as p1,
            tc.tile_pool(name="p1s", bufs=1) as p1s,
            tc.tile_pool(name="p1ps", bufs=2, space="PSUM") as p1ps,
            tc.tile_pool(name="p1rb", bufs=1, space="PSUM") as p1rb,
            tc.tile_pool(name="p1pst", bufs=2, space="PSUM") as p1pst,
        ):
            xc = {}
            xc[0] = xch.tile([128, ET, CHT], BF16, tag="xc", name="xc0")
            nc.sync.dma_start(out=xc[0], in_=xt_bf[0])
            wqkv_sb = p1s.tile([128, ET, 3, 128], BF16, tag="wqkv")
            nc.sync.dma_start(out=wqkv_sb, in_=wqkv_d[:])
            for c in range(1, NCH):
                xc[c] = xch.tile([128, ET, CHT], BF16, tag="xc", name=f"xc{c}")
                nc.sync.dma_start(out=xc[c], in_=xt_bf[c])

            for tci in range(T // 512):
                ch, off = tci, 0

                def xs(e, _c=ch, _o=off):
                    return xc[_c][:, e, _o:_o + 512]

                c0 = 512 * tci
                # squares (fp8, x8) split ACT/DVE; sum via ones8 DoubleRow
                sq8 = p1.tile([128, ET, 512], FP8, tag="sq8")
                for e in range(ET):
                    if e % 2 == 0:
                        nc.scalar.activation(
                            sq8[:, e, :], xs(e), AF.Square,
                            scale=float(np.sqrt(SQ)),
                        )
                    else:
                        nc.vector.scalar_tensor_tensor(
                            out=sq8[:, e, :], in0=xs(e), scalar=SQ,
                            in1=xs(e), op0=ALU.mult, op1=ALU.mult,
                        )
                ss_ps = p1ps.tile([1, 512], F32, tag="ss")
                for i in range(ET // 2):
                    nc.tensor.matmul(
                        ss_ps[:], ones8_sb[:, :, 0:1], sq8[:, 2 * i:2 * i + 2, :],
                        start=(i == 0), stop=(i == ET // 2 - 1), perf_mode=DR,
                    )
                rinv = p1.tile([1, 512], F32, tag="rinv")
                nc.vector.reciprocal_approx_fast(out=rinv[:], in_=ss_ps[:])
                r = p1.tile([1, 512], F32R, tag="r")
                nc.scalar.activation(r[:], rinv[:], AF.Sqrt, scale=float(E * SQ))
                rb_ps = p1rb.tile([128, 512], F32, tag="rbps")
                nc.tensor.matmul(rb_ps[:], onesrow_sb[:], r[:], start=True, stop=True)
                rb = p1.tile([128, 512], F32R, tag="rb")
                nc.scalar.activation(rb[:], rb_ps[:], AF.Copy)

                # Q / K projections -> qhat/khat (rms scale deferred, fused w/ bias)
                for proj in range(2):
                    p_ps = p1ps.tile([128, 512], F32, tag="qkv", bufs=3)
                    for e in range(ET):
                        nc.tensor.matmul(
                            p_ps[:], wqkv_sb[:, e, proj, :], xs(e),
                            start=(e == 0), stop=(e == ET - 1),
                        )
                    dst = qhat if proj == 0 else khat
                    for hh in range(2):
                        nc.vector.scalar_tensor_tensor(
                            out=dst[hh][0:64, c0:c0 + 512],
                            in0=p_ps[64 * hh:64 * hh + 64, :],
                            scalar=bqkv_sb[64 * hh:64 * hh + 64, proj:proj + 1],
                            in1=rb[64 * hh:64 * hh + 64, :],
                            op0=ALU.add, op1=ALU.mult,
                        )
                # V -> natural layout via transpose; fp8 x SA
                v_ps = p1ps.tile([128, 512], F32, tag="qkv", bufs=3)
                for e in range(ET):
                    nc.tensor.matmul(
                        v_ps[:], wqkv_sb[:, e, 2, :], xs(e),
                        start=(e == 0), stop=(e == ET - 1),
                    )
                vt_sb = p1.tile([128, 512], F32R, tag="vt")
                nc.vector.scalar_tensor_tensor(
                    out=vt_sb[:], in0=v_ps[:], scalar=bqkv_sb[:, 2:3],
                    in1=rb[:], op0=ALU.add, op1=ALU.mult,
                )
                for j in range(4):
                    vtr_ps = p1pst.tile([128, 128], F32R, tag="vtr")
                    nc.tensor.transpose(
                        vtr_ps[:], vt_sb[:, 128 * j:128 * j + 128], ident_sb[:]
                    )
                    jt = 4 * tci + j
                    for hh in range(2):
                        nc.vector.tensor_scalar(
                            out=vnat[:, jt // 2, hh, jt % 2, 0:64],
                            in0=vtr_ps[:, 64 * hh:64 * hh + 64],
                            scalar1=SA, scalar2=None, op0=ALU.mult,
                        )

        # prefetch p3 weights (runs on DMA during attention)
        wo_sb = [p3w.tile([128, 2, 2, ET, 128], FP8, name=f"wo{h}", tag=f"wo{h}")
                 for h in range(2)]
        nc.sync.dma_start(out=wo_sb[0], in_=wo_d[0])
        nc.sync.dma_start(out=wo_sb[1], in_=wo_d[1])
        xo_sb = p3w.tile([128, ET, TOK], BF16, tag="xo")
        nc.sync.dma_start(out=xo_sb, in_=xt_own[:])

        # ================= Phase 2: attention (head-outer) =================
        with (
            tc.tile_pool(name="p2a", bufs=4) as p2a,
            tc.tile_pool(name="p2b", bufs=2) as p2b,
            tc.tile_pool(name="p2sp", bufs=2, space="PSUM") as p2sp,
            tc.tile_pool(name="p2sd", bufs=1, space="PSUM") as p2sd,
            tc.tile_pool(name="p2c", bufs=2, space="PSUM") as p2c,
        ):
            for hh in range(2):
                for s in range(B):
                    for qp in range(NQC):
                        q0 = s * L + QC * qp
                        dst = 4 * s + qp
                        ctx_ps = p2c.tile([65, QC], F32, tag="ctx")
                        # unmasked full k-tile pairs (kt < 4qp)
                        for pr in range(2 * qp):
                            sp = p2sp.tile([128, 2, QC], F32, tag="sp")
                            for i in range(2):
                                kt = 2 * pr + i
                                koff = s * L + 128 * kt
                                nc.tensor.matmul(
                                    sp[:, i, :], khat[hh][:, koff:koff + 128],
                                    qhat[hh][:, q0:q0 + QC],
                                    start=True, stop=True,
                                )
                            ap_t = p2a.tile([128, 2, QC], FP8, tag="ap")
                            nc.scalar.activation(ap_t[:, :, :], sp[:, :, :], AF.Exp)
                            nc.tensor.matmul(
                                ctx_ps[:, :], vnat[:, 8 * s + pr, hh, :, 0:65],
                                ap_t[:, :, :],
                                start=(pr == 0), stop=False, perf_mode=DR,
                            )
                        # straddle pair kt = 4qp, 4qp+1 (full width, masked)
                        sp = p2sp.tile([128, 2, QC], F32, tag="sp")
                        for i in range(2):
                            kt = 4 * qp + i
                            koff = s * L + 128 * kt
                            nc.tensor.matmul(
                                sp[:, i, :], khat[hh][:, koff:koff + 128],
                                qhat[hh][:, q0:q0 + QC],
                                start=True, stop=True,
                            )
                        scl = p2b.tile([128, 2, QC], F32R, tag="scl")
                        nc.vector.tensor_add(scl[:, :, :], sp[:, :, :], mbs_sb[:, :, :])
                        ap_t = p2a.tile([128, 2, QC], FP8, tag="ap")
                        nc.scalar.activation(ap_t[:, :, :], scl[:, :, :], AF.Exp)
                        nc.tensor.matmul(
                            ctx_ps[:, :], vnat[:, 8 * s + 2 * qp, hh, :, 0:65],
                            ap_t[:, :, :],
                            start=(qp == 0), stop=False, perf_mode=DR,
                        )
                        # diagonal pair kt = 4qp+2, 4qp+3 (half width, masked)
                        sd = p2sd.tile([128, 2, QC], F32, tag="sd")
                        for i in range(2):
                            kt = 4 * qp + 2 + i
                            koff = s * L + 128 * kt
                            nc.tensor.matmul(
                                sd[:, i, 0:QC // 2], khat[hh][:, koff:koff + 128],
                                qhat[hh][:, q0 + QC // 2:q0 + QC],
                                start=True, stop=True,
                            )
                        scl_d = p2b.tile([128, 2, QC // 2], F32R, tag="scld")
                        nc.vector.tensor_add(
                            scl_d[:, :, :], sd[:, :, 0:QC // 2], mbd_sb[:, :, :]
                        )
                        ap_d = p2a.tile([128, 2, QC // 2], FP8, tag="apd")
                        nc.scalar.activation(ap_d[:, :, :], scl_d[:, :, :], AF.Exp)
                        nc.tensor.matmul(
                            ctx_ps[:, QC // 2:QC],
                            vnat[:, 8 * s + 2 * qp + 1, hh, :, 0:65],
                            ap_d[:, :, :],
                            start=False, stop=True, perf_mode=DR,
                        )
                        # normalize -> fp8 ctx chunk, straight to a2a staging
                        rs_sb = p2b.tile([1, QC], F32, tag="rssb")
                        nc.vector.tensor_copy(rs_sb[:], ctx_ps[64:65, :])
                        rs_inv = p2b.tile([1, QC], F32, tag="rsinv")
                        nc.vector.reciprocal_approx_fast(
                            out=rs_inv[:], in_=rs_sb[:]
                        )
                        rb_a = p2b.tile([64, QC], F32, tag="rba")
                        nc.gpsimd.partition_broadcast(rb_a[:], rs_inv[:])
                        cxc = p2a.tile([64, QC], FP8, tag="cxc")
                        nc.vector.tensor_mul(cxc[:], ctx_ps[0:64, :], rb_a[:])
                        nc.sync.dma_start(out=a2a_in[hh][dst], in_=cxc[:])
                nc.gpsimd.collective_compute(
                    "AllToAll", ALU.bypass,
                    replica_groups=[list(range(NC_))],
                    ins=[a2a_in[hh][:]], outs=[a2a_out[hh][:]],
                )

        att_stack.close()

        # wout prefetch (after attention SBUF is freed)
        wout_sb = p3w.tile([128, DT // 2, 2, ET, 128], FP8, tag="wout")
        nc.sync.dma_start(out=wout_sb, in_=wout_d[:])

        # ================= Phase 3a: out-proj + residual + cross stage ========
        x_stack = ExitStack()
        x_pool = x_stack.enter_context(tc.tile_pool(name="xp", bufs=1))
        x2 = x_pool.tile([128, ET, TOK], F32R, tag="x2")
        x3 = x_pool.tile([128, ET, TOK], F32R, tag="x3")
        h3 = x_pool.tile([128, ET, TOK], FP8, tag="h3")

        with (
            tc.tile_pool(name="p3c", bufs=1) as p3c,
            tc.tile_pool(name="p3op", bufs=1, space="PSUM") as p3op,
        ):
            cxt = [p3c.tile([128, 4, TOK], FP8, name=f"cxt{h}", tag=f"cxt{h}")
                   for h in range(2)]
            op_ps = [p3op.tile([128, TOK], F32, name=f"op{e}", tag=f"op{e}")
                     for e in range(ET)]
            for h2 in range(2):
                for kt in range(4):
                    nc.sync.dma_start(
                        out=cxt[h2][0:64, kt, :], in_=a2a_out[h2][2 * kt],
                    )
                    nc.sync.dma_start(
                        out=cxt[h2][64:128, kt, :], in_=a2a_out[h2][2 * kt + 1],
                    )
                for e in range(ET):
                    for j in range(2):
                        nc.tensor.matmul(
                            op_ps[e][:], wo_sb[h2][:, j, :, e, :],
                            cxt[h2][:, 2 * j:2 * j + 2, :],
                            start=(h2 == 0 and j == 0), stop=(h2 == 1 and j == 1),
                            perf_mode=DR,
                        )
            for e in range(ET):
                nc.vector.scalar_tensor_tensor(
                    out=x2[:, e, :], in0=op_ps[e][:], scalar=1.0 / SWA,
                    in1=xo_sb[:, e, :], op0=ALU.mult, op1=ALU.add,
                )

        def rms_scale(src, tag, extra_scale, pool, pool_ps):
            """sum-of-squares over e-tiles of src -> broadcast scale tile."""
            sq8 = pool.tile([128, ET, TOK], FP8, tag=f"sq{tag}")
            for e in range(ET):
                if e % 2 == 0:
                    nc.scalar.activation(
                        sq8[:, e, :], src[:, e, :], AF.Square,
                        scale=float(np.sqrt(SQ)),
                    )
                else:
                    nc.vector.scalar_tensor_tensor(
                        out=sq8[:, e, :], in0=src[:, e, :], scalar=SQ,
                        in1=src[:, e, :], op0=ALU.mult, op1=ALU.mult,
                    )
            ss_ps = pool_ps.tile([1, TOK], F32, tag=f"ss{tag}")
            for i in range(ET // 2):
                nc.tensor.matmul(
                    ss_ps[:], ones8_sb[:, :, 0:1], sq8[:, 2 * i:2 * i + 2, :],
                    start=(i == 0), stop=(i == ET // 2 - 1), perf_mode=DR,
                )
            rinv = pool.tile([1, TOK], F32, tag=f"ri{tag}")
            nc.vector.reciprocal_approx_fast(out=rinv[:], in_=ss_ps[:])
            r = pool.tile([1, TOK], F32R, tag=f"r{tag}")
            nc.scalar.activation(
                r[:], rinv[:], AF.Sqrt, scale=float(E * SQ * extra_scale)
            )
            return r

        with (
            tc.tile_pool(name="p3n", bufs=2) as p3n,
            tc.tile_pool(name="p3nps", bufs=2, space="PSUM") as p3nps,
        ):
            r2 = rms_scale(x2, "2", 1.0, p3n, p3nps)
            r2p = p3n.tile([1, TOK], F32R, tag="r2p")
            nc.vector.tensor_scalar(
                out=r2p[:], in0=r2[:], scalar1=1.0, scalar2=None, op0=ALU.add
            )
            rb2_ps = p3nps.tile([128, TOK], F32, tag="rb2ps")
            nc.tensor.matmul(rb2_ps[:], onesrow_sb[:], r2p[:], start=True, stop=True)
            rb2 = p3n.tile([128, TOK], F32R, tag="rb2")
            nc.scalar.activation(rb2[:], rb2_ps[:], AF.Copy)
            for e in range(ET):
                nc.vector.tensor_mul(x3[:, e, :], x2[:, e, :], rb2[:])

            r3 = rms_scale(x3, "3", SA * SA, p3n, p3nps)
            rb3_ps = p3nps.tile([128, TOK], F32, tag="rb2ps")
            nc.tensor.matmul(rb3_ps[:], onesrow_sb[:], r3[:], start=True, stop=True)
            rb3 = p3n.tile([128, TOK], F32R, tag="rb3")
            nc.scalar.activation(rb3[:], rb3_ps[:], AF.Copy)
            for e in range(ET):
                nc.vector.tensor_mul(h3[:, e, :], x3[:, e, :], rb3[:])

        # ================= Phase 3b: GLU FFN =================
        with (
            tc.tile_pool(name="g", bufs=1) as gpool,
            tc.tile_pool(name="f3w", bufs=4) as f3w,
            tc.tile_pool(name="f3o", bufs=3) as f3o,
        ):
            g = gpool.tile([128, DT, TOK], FP8, tag="g")
            uv_ps_pool = ExitStack()
            f3ps = uv_ps_pool.enter_context(
                tc.tile_pool(name="f3ps", bufs=4, space="PSUM")
            )
            for d in range(DT):
                wwv_c = f3w.tile([128, 2, 4, 2, 128], FP8, tag="wwvc")
                nc.sync.dma_start(out=wwv_c, in_=wwv_d[d])
                u_ps = f3ps.tile([128, TOK], F32, tag="mm")
                for i in range(ET // 2):
                    nc.tensor.matmul(
                        u_ps[:], wwv_c[:, 0, i, :, :], h3[:, 2 * i:2 * i + 2, :],
                        start=(i == 0), stop=(i == ET // 2 - 1), perf_mode=DR,
                    )
                v_ps = f3ps.tile([128, TOK], F32, tag="mm")
                for i in range(ET // 2):
                    nc.tensor.matmul(
                        v_ps[:], wwv_c[:, 1, i, :, :], h3[:, 2 * i:2 * i + 2, :],
                        start=(i == 0), stop=(i == ET // 2 - 1), perf_mode=DR,
                    )
                gl = f3o.tile([128, TOK], BF16, tag="gl")
                nc.scalar.activation(gl[:], u_ps[:], AF.Gelu, scale=1.0 / SWA)
                nc.vector.scalar_tensor_tensor(
                    out=g[:, d, :], in0=v_ps[:], scalar=SA / SWA, in1=gl[:],
                    op0=ALU.mult, op1=ALU.mult,
                )
            uv_ps_pool.close()

            # Wout + residual, e-outer, streamed output
            with tc.tile_pool(name="fps2", bufs=2, space="PSUM") as fps2:
                for e in range(ET):
                    f_ps = fps2.tile([128, TOK], F32, tag="fps")
                    for dp in range(DT // 2):
                        nc.tensor.matmul(
                            f_ps[:], wout_sb[:, dp, :, e, :],
                            g[:, 2 * dp:2 * dp + 2, :],
                            start=(dp == 0), stop=(dp == DT // 2 - 1),
                            perf_mode=DR,
                        )
                    y_sb = f3o.tile([128, TOK], BF16, tag="ysb")
                    nc.vector.scalar_tensor_tensor(
                        out=y_sb[:], in0=f_ps[:], scalar=1.0 / SWA,
                        in1=x3[:, e, :], op0=ALU.mult, op1=ALU.add,
                    )
                    nc.sync.dma_start(out=y[e], in_=y_sb[:])

        x_stack.close()
        p3w_stack.close()
        estack.close()

    nc.finalize()
    return nc


def make_in_maps(X, Wqkv, bqkv_in, Wo_sa, bo_sa, Ww, Wv, Wout):
    bf = ml_dtypes.bfloat16
    f8 = ml_dtypes.float8_e4m3
    Xf = np.ascontiguousarray(np.asarray(X, np.float32).reshape(T, E))
    XT = np.ascontiguousarray(Xf.T)  # [E, T]
    # chunk-major [8, 128, ET, 512]: contiguous 8KB per partition per chunk
    xt_bf = np.ascontiguousarray(
        XT.reshape(ET, 128, 8, T // 8).transpose(2, 1, 0, 3).astype(bf)
    )
    bo = np.asarray(bo_sa, np.float32)
    xt_own_all = XT + bo[:, None]  # [E, T] f32 with bias folded
    Wr = np.asarray(Wqkv, np.float32).reshape(E, H, 3, D)
    br = np.asarray(bqkv_in, np.float32).reshape(H, 3, D)
    pos = (np.arange(T, dtype=np.float32) % L)
    slopes_all = (2.0 ** (-np.linspace(1.0, 8.0, H))).astype(np.float32)
    ones_col = np.ones([128, 1], np.float32)
    ones8 = np.ones([128, 2, 16], f8)
    onesrow = np.ones([1, 128], np.float32)
    ones_t = np.ones([T], np.float32)

    def ext_hilo(h0):
        """[2, 4, T] bf16 hi/lo-split alibi rows for heads h0, h0+1."""
        ek = np.empty((2, 4, T), np.float32)
        eq = np.empty((2, 4, T), np.float32)
        for i, h in enumerate((h0, h0 + 1)):
            v = slopes_all[h] * pos
            hi = v.astype(bf).astype(np.float32)
            lo = v - hi
            ek[i] = np.stack([hi, lo, ones_t, ones_t])
            eq[i] = np.stack([ones_t, ones_t, -hi, -lo])
        return ek.astype(bf), eq.astype(bf)
    p_i = np.arange(128)[:, None, None]
    i_i = np.arange(2)[None, :, None]
    f_s = np.arange(QC)[None, None, :]
    mbs = np.where(f_s >= p_i + 128 * i_i, 0.0, -30000.0).astype(np.float32)
    f_d = np.arange(QC // 2)[None, None, :]
    mbd = np.where(f_d >= p_i + 128 * i_i, 0.0, -30000.0).astype(np.float32)
    ident = np.eye(128, dtype=np.float32)
    vones = np.ones([128, 64], f8)

    # out-proj fp8 x SW: wo_d[h2, p, ktp, j, e, m] =
    #   Wo[((2*(2ktp+j) + p//64)*2 + h2)*64 + p%64, e*128+m]
    Wo = np.asarray(Wo_sa, np.float32) * SW
    wo_t = np.empty((2, 128, 2, 2, ET, 128), np.float32)
    Wo5 = Wo.reshape(NC_, 2, 64, ET, 128)  # [src, hsel, d, e, m]
    for h2 in range(2):
        w = Wo5[:, h2]                      # [8 src, 64, e, m]
        # p = src_rel*64 + d; src = 2*(2ktp+j) + src_rel
        w = w.reshape(2, 2, 2, 64, ET, 128)  # [ktp, j, src_rel, d, e, m]
        wo_t[h2] = w.transpose(2, 3, 0, 1, 4, 5).reshape(128, 2, 2, ET, 128)
    wo_t8 = np.ascontiguousarray(wo_t.astype(f8))

    # wwv fp8 x SW: [DT, 128, 2(uv), 4(ep), 2(j), 128]
    W2 = np.stack([np.asarray(Ww, np.float32), np.asarray(Wv, np.float32)], 0) * SW
    # [uv, e, p, d, m] -> [d, p, uv, ep, j, m] with e = 2*ep + j
    wwv = W2.reshape(2, ET // 2, 2, 128, DT, 128).transpose(4, 3, 0, 1, 2, 5)
    wwv_t8 = np.ascontiguousarray(wwv.astype(f8))

    # wout fp8 x SW: [128, DT//2, 2, ET, 128]; k-row = (2*dp + j)*128 + p
    Wo2 = np.asarray(Wout, np.float32) * SW
    wout = Wo2.reshape(DT // 2, 2, 128, ET, 128).transpose(2, 0, 1, 3, 4)
    wout_t8 = np.ascontiguousarray(wout.astype(f8))

    in_maps = []
    for c in range(NC_):
        h0 = 2 * c
        extk, extq = ext_hilo(h0)
        wq_p = np.concatenate([Wr[:, h0, 0, :], Wr[:, h0 + 1, 0, :]], axis=1) * 0.125
        wk_p = np.concatenate([Wr[:, h0, 1, :], Wr[:, h0 + 1, 1, :]], axis=1)
        wv_p = np.concatenate([Wr[:, h0, 2, :], Wr[:, h0 + 1, 2, :]], axis=1)
        wqkv_t = np.stack([wq_p, wk_p, wv_p], axis=1)      # [E, 3, 128]
        wqkv_t = np.ascontiguousarray(
            wqkv_t.reshape(ET, 128, 3, 128).transpose(1, 0, 2, 3).astype(bf)
        )  # [128, ET, 3, 128]
        bq_p = np.concatenate([br[h0, 0], br[h0 + 1, 0]]) * 0.125
        bk_p = np.concatenate([br[h0, 1], br[h0 + 1, 1]])
        bv_p = np.concatenate([br[h0, 2], br[h0 + 1, 2]])
        bq3 = np.stack([bq_p, bk_p, bv_p], axis=1).astype(np.float32)  # [128,3]
        xt_own = np.ascontiguousarray(
            xt_own_all[:, TOK * c:TOK * (c + 1)]
            .reshape(ET, 128, TOK).transpose(1, 0, 2).astype(bf)
        )
        in_maps.append({
            "xt_bf": xt_bf,
            "xt_own": xt_own,
            "wqkv_d": wqkv_t,
            "bqkv": bq3,
            "wo_d": wo_t8,
            "wwv_d": wwv_t8,
            "wout_d": wout_t8,
            "onesd": ones_col,
            "ones8d": ones8,
            "onesrowd": onesrow,
            "extkd": extk,
            "extqd": extq,
            "mbsd": mbs,
            "mbdd": mbd,
            "identd": ident,
            "vonesd": vones,
        })
    return in_maps


def kernel(**inputs) -> np.ndarray:
    out, _ = run(inputs, trace=False)
    return out


def run(inputs, trace=False):
    if "nc" not in _CACHE:
        _CACHE["nc"] = build_nc()
    nc = _CACHE["nc"]
    in_maps = make_in_maps(
        inputs["X"], inputs["Wqkv"], inputs["bqkv"], inputs["Wo_sa"],
        inputs["bo_sa"], inputs["Ww"], inputs["Wv"], inputs["Wout"],
    )
    if trace:
        install_ntff_hook()
    res = run_bass_kernel_spmd(nc, in_maps, list(range(NC_)), trace=trace)
    out = np.concatenate(
        [np.asarray(r["y"]).astype(np.float32).reshape(E, TOK)
         for r in res.results], axis=1
    )  # [E, T]
    return np.ascontiguousarray(out.T).reshape(B, L, E).astype(np.float32), res


# revision 27
# speedup vs baseline: 1.0695x; 1.0695x over previous
"""Trainium2 Bass kernel for nn_DecoderBlock (B=2, L=2048, E=1024, H=16, D=64, DFF=4096).

Strategy (8 NeuronCores, SPMD):
  - Attention head-parallel (2 heads/core), everything else token-parallel
    (512 own tokens/core).  Head->token reshard via one AllToAll per head
    (fp8 payload), first overlapped with second head's compute.
  - fp8e4 + DoubleRow matmuls for: attention probs x V, out-proj, GLU FFN
    (u/v/Wout).  QKV projection and QK^T stay bf16/f32r for accuracy.
  - exp on ACT with direct fp8 output; masked tiles get a DVE mask-add first.
  - rmsnorm sum-of-squares via fp8 squares + DoubleRow ones-contraction;
    partition-broadcast of the scale via a K=1 PE matmul + ACT copy
    (gpsimd.partition_broadcast is ~7.6us per [128,512] - too slow).
  - X loaded once as [128, ET, T] bf16 in 4 chunks (2KB DMA lines), phase 1
    consumes it straight from SBUF.
  - ctx chunks DMA'd straight into the AllToAll staging dram during
    attention; FFN weights prefetched during the second AllToAll.
  - Output written per e-tile in bf16 as soon as its Wout accumulation ends.
"""

import sys
import types

import numpy as np
import ml_dtypes

sys.path.insert(0, "/opt/trn_rl_repo")
sys.path.insert(0, "/opt/pypackages")

import concourse.bass as bass
import concourse.mybir as mybir
from concourse import bacc
import concourse.tile as tile
from concourse.bass_utils import run_bass_kernel_spmd

F32 = mybir.dt.float32
F32R = mybir.dt.float32r
BF16 = mybir.dt.bfloat16
FP8 = mybir.dt.float8e4
AF = mybir.ActivationFunctionType
ALU = mybir.AluOpType
DR = mybir.MatmulPerfMode.DoubleRow

B, L, E, H, D, DFF = 2, 2048, 1024, 16, 64, 4096
T = B * L            # 4096 flat tokens
NC_ = 8              # cores
TOK = T // NC_       # 512 own tokens per core
ET = E // 128        # 8 e-tiles
DT = DFF // 128      # 32 dff-tiles
QC = 512             # q-chunk width in attention
NQC = L // QC        # 4 q-chunks per sequence
SW = 64.0            # fp8 weight scale
SA = 16.0            # fp8 activation scale
SQ = 1.0             # fp8 squares scale (>1 overflows fp8: max x3^2*8 > 240)
SWA = SW * SA        # matmul output scale

_CACHE = {}


def install_ntff_hook():
    """Synthesize antenv.axon_hooks so trace=True can profile via libaxon_pjrt."""
    try:
        from antenv.axon_hooks import get_axon_ntff_profile_hook  # noqa
        return
    except ImportError:
        pass
    try:
        import antenv
        mod = types.ModuleType("antenv.axon_hooks")
        mod._hook = None
        mod.set_axon_ntff_profile_hook = lambda h: setattr(mod, "_hook", h)
        mod.get_axon_ntff_profile_hook = lambda: mod._hook
        sys.modules["antenv.axon_hooks"] = mod
        antenv.axon_hooks = mod
        if "/root/.axon_site" not in sys.path:
            sys.path.insert(0, "/root/.axon_site")
        from trn_agent_boot.trn_boot import _ntff_profile_via_ctypes
        mod.set_axon_ntff_profile_hook(
            _ntff_profile_via_ctypes("/opt/axon/libaxon_pjrt.so")
        )
    except Exception:
        pass


def build_nc():
    nc = bacc.Bacc("TRN2", target_bir_lowering=False, debug=False)

    # ---- I/O ----
    xt_bf = nc.dram_tensor("xt_bf", [8, 128, ET, T // 8], BF16, kind="ExternalInput")
    xt_own = nc.dram_tensor("xt_own", [128, ET, TOK], BF16, kind="ExternalInput")
    wqkv_d = nc.dram_tensor("wqkv_d", [128, ET, 3, 128], BF16, kind="ExternalInput")
    bqkv = nc.dram_tensor("bqkv", [128, 3], F32, kind="ExternalInput")
    wo_d = nc.dram_tensor("wo_d", [2, 128, 2, 2, ET, 128], FP8, kind="ExternalInput")
    wwv_d = nc.dram_tensor("wwv_d", [DT, 128, 2, 4, 2, 128], FP8, kind="ExternalInput")
    wout_d = nc.dram_tensor("wout_d", [128, DT // 2, 2, ET, 128], FP8, kind="ExternalInput")
    onesd = nc.dram_tensor("onesd", [128, 1], F32R, kind="ExternalInput")
    ones8d = nc.dram_tensor("ones8d", [128, 2, 16], FP8, kind="ExternalInput")
    onesrowd = nc.dram_tensor("onesrowd", [1, 128], F32R, kind="ExternalInput")
    extkd = nc.dram_tensor("extkd", [2, 4, T], BF16, kind="ExternalInput")
    extqd = nc.dram_tensor("extqd", [2, 4, T], BF16, kind="ExternalInput")
    mbsd = nc.dram_tensor("mbsd", [128, 2, QC], F32R, kind="ExternalInput")
    mbdd = nc.dram_tensor("mbdd", [128, 2, QC // 2], F32R, kind="ExternalInput")
    identd = nc.dram_tensor("identd", [128, 128], F32R, kind="ExternalInput")
    vonesd = nc.dram_tensor("vonesd", [128, 64], FP8, kind="ExternalInput")

    y = nc.dram_tensor("y", [ET, 128, TOK], BF16, kind="ExternalOutput")

    a2a_in = [nc.dram_tensor(f"a2a_in{h}", [NC_, 64, TOK], FP8) for h in range(2)]
    a2a_out = [nc.dram_tensor(f"a2a_out{h}", [NC_, 64, TOK], FP8) for h in range(2)]
    a2aw_in = nc.dram_tensor("a2aw_in", [NC_, 16], FP8)
    a2aw_out = nc.dram_tensor("a2aw_out", [NC_, 16], FP8)

    with tile.TileContext(nc) as tc:
        from contextlib import ExitStack
        estack = ExitStack()
        const = estack.enter_context(tc.tile_pool(name="const", bufs=1))
        ones_sb = const.tile([128, 1], F32R)
        nc.sync.dma_start(out=ones_sb, in_=onesd[:])
        ones8_sb = const.tile([128, 2, 16], FP8, tag="ones8")
        nc.sync.dma_start(out=ones8_sb, in_=ones8d[:])
        onesrow_sb = const.tile([1, 128], F32R, tag="onesrow")
        nc.sync.dma_start(out=onesrow_sb, in_=onesrowd[:])
        mbs_sb = const.tile([128, 2, QC], F32R, tag="mbs")
        nc.sync.dma_start(out=mbs_sb, in_=mbsd[:])
        mbd_sb = const.tile([128, 2, QC // 2], F32R, tag="mbd")
        nc.sync.dma_start(out=mbd_sb, in_=mbdd[:])
        ident_sb = const.tile([128, 128], F32R)
        nc.sync.dma_start(out=ident_sb, in_=identd[:])
        bqkv_sb = const.tile([128, 3], F32)
        nc.sync.dma_start(out=bqkv_sb, in_=bqkv[:])

        # phase-3 weight pool (opened early for pool stack order; DMAs are
        # emitted after phase 1 so they run during attention)
        p3w_stack = ExitStack()
        p3w = p3w_stack.enter_context(tc.tile_pool(name="p3w", bufs=1))

        # ---------- persistent attention state ----------
        att_stack = ExitStack()
        att_pool = att_stack.enter_context(tc.tile_pool(name="att", bufs=1))
        qhat = [att_pool.tile([68, T], BF16, name=f"qhat{h}", tag=f"qhat{h}") for h in range(2)]
        khat = [att_pool.tile([68, T], BF16, name=f"khat{h}", tag=f"khat{h}") for h in range(2)]
        # V natural layout: [128 k-tokens, jp(=T//256), hh, kt-in-pair, 80]
        # col 64 = ones (rowsum trick), cols 65..79 pad for step%16==0
        vnat = att_pool.tile([128, T // 256, 2, 2, 80], FP8, tag="vnat")
        vones_sb = att_pool.tile([128, T // 256, 2, 2, 1], FP8, tag="vones")
        nc.sync.dma_start(out=vones_sb, in_=vonesd[:])
        nc.vector.tensor_copy(vnat[:, :, :, :, 64:65], vones_sb[:])

        # alibi extension rows (hi/lo split, precomputed host-side)
        for hh in range(2):
            nc.sync.dma_start(out=qhat[hh][64:68, :], in_=extqd[hh])
            nc.sync.dma_start(out=khat[hh][64:68, :], in_=extkd[hh])
        # tiny warm-up collective so the real A2As skip the ncfw cold start
        nc.gpsimd.collective_compute(
            "AllToAll", ALU.bypass, replica_groups=[list(range(NC_))],
            ins=[a2aw_in[:]], outs=[a2aw_out[:]],
        )

        # ================= Phase 1: rmsnorm1 + QKV projections =================
        NCH = 8                      # X load chunks
        CHT = T // NCH               # tokens per chunk
        with (
            tc.tile_pool(name="xch", bufs=2) as xch,
            tc.tile_pool(name="p1", bufs=2) as p1,
            tc.tile_pool(name="p1s", bufs=1) as p1s,
            tc.tile_pool(name="p1ps", bufs=2, space="PSUM") as p1ps,
            tc.tile_pool(name="p1rb", bufs=1, space="PSUM") as p1rb,
            tc.tile_pool(name="p1pst", bufs=2, space="PSUM") as p1pst,
        ):
            xc = {}
            xc[0] = xch.tile([128, ET, CHT], BF16, tag="xc", name="xc0")
            nc.sync.dma_start(out=xc[0], in_=xt_bf[0])
            wqkv_sb = p1s.tile([128, ET, 3, 128], BF16, tag="wqkv")
            nc.sync.dma_start(out=wqkv_sb, in_=wqkv_d[:])
            for c in range(1, NCH):
                xc[c] = xch.tile([128, ET, CHT], BF16, tag="xc", name=f"xc{c}")
                nc.sync.dma_start(out=xc[c], in_=xt_bf[c])

            for tci in range(T // 512):
                ch, off = tci, 0

                def xs(e, _c=ch, _o=off):
                    return xc[_c][:, e, _o:_o + 512]

                c0 = 512 * tci
                # squares (fp8, x8) split ACT/DVE; sum via ones8 DoubleRow
                sq8 = p1.tile([128, ET, 512], FP8, tag="sq8")
                for e in range(ET):
                    if e % 2 == 0:
                        nc.scalar.activation(
                            sq8[:, e, :], xs(e), AF.Square,
                            scale=float(np.sqrt(SQ)),
                        )
                    else:
                        nc.vector.scalar_tensor_tensor(
                            out=sq8[:, e, :], in0=xs(e), scalar=SQ,
                            in1=xs(e), op0=ALU.mult, op1=ALU.mult,
                        )
                ss_ps = p1ps.tile([1, 512], F32, tag="ss")
                for i in range(ET // 2):
                    nc.tensor.matmul(
                        ss_ps[:], ones8_sb[:, :, 0:1], sq8[:, 2 * i:2 * i + 2, :],
                        start=(i == 0), stop=(i == ET // 2 - 1), perf_mode=DR,
                    )
                rinv = p1.tile([1, 512], F32, tag="rinv")
                nc.vector.reciprocal_approx_fast(out=rinv[:], in_=ss_ps[:])
                r = p1.tile([1, 512], F32R, tag="r")
                nc.scalar.activation(r[:], rinv[:], AF.Sqrt, scale=float(E * SQ))
                rb_ps = p1rb.tile([128, 512], F32, tag="rbps")
                nc.tensor.matmul(rb_ps[:], onesrow_sb[:], r[:], start=True, stop=True)
                rb = p1.tile([128, 512], F32R, tag="rb")
                nc.scalar.activation(rb[:], rb_ps[:], AF.Copy)

                # Q / K projections -> qhat/khat (rms scale deferred, fused w/ bias)
                for proj in range(2):
                    p_ps = p1ps.tile([128, 512], F32, tag="qkv", bufs=3)
                    for e in range(ET):
                        nc.tensor.matmul(
                            p_ps[:], wqkv_sb[:, e, proj, :], xs(e),
                            start=(e == 0), stop=(e == ET - 1),
                        )
                    dst = qhat if proj == 0 else khat
                    for hh in range(2):
                        nc.vector.scalar_tensor_tensor(
                            out=dst[hh][0:64, c0:c0 + 512],
                            in0=p_ps[64 * hh:64 * hh + 64, :],
                            scalar=bqkv_sb[64 * hh:64 * hh + 64, proj:proj + 1],
                            in1=rb[64 * hh:64 * hh + 64, :],
                            op0=ALU.add, op1=ALU.mult,
                        )
                # V -> natural layout via transpose; fp8 x SA
                v_ps = p1ps.tile([128, 512], F32, tag="qkv", bufs=3)
                for e in range(ET):
                    nc.tensor.matmul(
                        v_ps[:], wqkv_sb[:, e, 2, :], xs(e),
                        start=(e == 0), stop=(e == ET - 1),
                    )
                vt_sb = p1.tile([128, 512], F32R, tag="vt")
                nc.vector.scalar_tensor_tensor(
                    out=vt_sb[:], in0=v_ps[:], scalar=bqkv_sb[:, 2:3],
                    in1=rb[:], op0=ALU.add, op1=ALU.mult,
                )
                for j in range(4):
                    vtr_ps = p1pst.tile([128, 128], F32R, tag="vtr")
                    nc.tensor.transpose(
                        vtr_ps[:], vt_sb[:, 128 * j:128 * j + 128], ident_sb[:]
                    )
                    jt = 4 * tci + j
                    for hh in range(2):
                        nc.vector.tensor_scalar(
                            out=vnat[:, jt // 2, hh, jt % 2, 0:64],
                            in0=vtr_ps[:, 64 * hh:64 * hh + 64],
                            scalar1=SA, scalar2=None, op0=ALU.mult,
                        )

        # prefetch p3 weights (runs on DMA during attention)
        wo_sb = [p3w.tile([128, 2, 2, ET, 128], FP8, name=f"wo{h}", tag=f"wo{h}")
                 for h in range(2)]
        nc.sync.dma_start(out=wo_sb[0], in_=wo_d[0])
        nc.sync.dma_start(out=wo_sb[1], in_=wo_d[1])
        xo_sb = p3w.tile([128, ET, TOK], BF16, tag="xo")
        nc.sync.dma_start(out=xo_sb, in_=xt_own[:])

        # ================= Phase 2: attention (head-outer) =================
        with (
            tc.tile_pool(name="p2a", bufs=6) as p2a,
            tc.tile_pool(name="p2b", bufs=2) as p2b,
            tc.tile_pool(name="p2sp", bufs=3, space="PSUM") as p2sp,
            tc.tile_pool(name="p2c", bufs=2, space="PSUM") as p2c,
        ):
            for hh in range(2):
                for s in range(B):
                    for qp in range(NQC):
                        q0 = s * L + QC * qp
                        dst = 4 * s + qp
                        ctx_ps = p2c.tile([65, QC], F32, tag="ctx")
                        # unmasked full k-tile pairs (kt < 4qp)
                        for pr in range(2 * qp):
                            sp = p2sp.tile([128, 2, QC], F32, tag="sp")
                            for i in range(2):
                                kt = 2 * pr + i
                                koff = s * L + 128 * kt
                                nc.tensor.matmul(
                                    sp[:, i, :], khat[hh][:, koff:koff + 128],
                                    qhat[hh][:, q0:q0 + QC],
                                    start=True, stop=True,
                                )
                            ap_t = p2a.tile([128, 2, QC], FP8, tag="ap")
                            nc.scalar.activation(ap_t[:, :, :], sp[:, :, :], AF.Exp)
                            nc.tensor.matmul(
                                ctx_ps[:, :], vnat[:, 8 * s + pr, hh, :, 0:65],
                                ap_t[:, :, :],
                                start=(pr == 0), stop=False, perf_mode=DR,
                            )
                        # straddle pair kt = 4qp, 4qp+1 (full width, masked)
                        sp = p2sp.tile([128, 2, QC], F32, tag="sp")
                        for i in range(2):
                            kt = 4 * qp + i
                            koff = s * L + 128 * kt
                            nc.tensor.matmul(
                                sp[:, i, :], khat[hh][:, koff:koff + 128],
                                qhat[hh][:, q0:q0 + QC],
                                start=True, stop=True,
                            )
                        scl = p2b.tile([128, 2, QC], F32R, tag="scl")
                        nc.vector.tensor_add(scl[:, :, :], sp[:, :, :], mbs_sb[:, :, :])
                        ap_t = p2a.tile([128, 2, QC], FP8, tag="ap")
                        nc.scalar.activation(ap_t[:, :, :], scl[:, :, :], AF.Exp)
                        nc.tensor.matmul(
                            ctx_ps[:, :], vnat[:, 8 * s + 2 * qp, hh, :, 0:65],
                            ap_t[:, :, :],
                            start=(qp == 0), stop=False, perf_mode=DR,
                        )
                        # diagonal pair kt = 4qp+2, 4qp+3 (half width, masked)
                        sd = p2sp.tile([128, 2, QC], F32, tag="sp", name="sd")
                        for i in range(2):
                            kt = 4 * qp + 2 + i
                            koff = s * L + 128 * kt
                            nc.tensor.matmul(
                                sd[:, i, 0:QC // 2], khat[hh][:, koff:koff + 128],
                                qhat[hh][:, q0 + QC // 2:q0 + QC],
                                start=True, stop=True,
                            )
                        scl_d = p2b.tile([128, 2, QC // 2], F32R, tag="scld")
                        nc.vector.tensor_add(
                            scl_d[:, :, :], sd[:, :, 0:QC // 2], mbd_sb[:, :, :]
                        )
                        ap_d = p2a.tile([128, 2, QC // 2], FP8, tag="apd")
                        nc.scalar.activation(ap_d[:, :, :], scl_d[:, :, :], AF.Exp)
                        nc.tensor.matmul(
                            ctx_ps[:, QC // 2:QC],
                            vnat[:, 8 * s + 2 * qp + 1, hh, :, 0:65],
                            ap_d[:, :, :],
                            start=False, stop=True, perf_mode=DR,
                        )
                        # normalize -> fp8 ctx chunk, straight to a2a staging
                        rs_sb = p2b.tile([1, QC], F32, tag="rssb")
                        nc.vector.tensor_copy(rs_sb[:], ctx_ps[64:65, :])
                        rs_inv = p2b.tile([1, QC], F32, tag="rsinv")
                        nc.vector.reciprocal_approx_fast(
                            out=rs_inv[:], in_=rs_sb[:]
                        )
                        rb_a = p2b.tile([64, QC], F32, tag="rba")
                        nc.gpsimd.partition_broadcast(rb_a[:], rs_inv[:])
                        cxc = p2a.tile([64, QC], FP8, tag="cxc")
                        nc.vector.tensor_mul(cxc[:], ctx_ps[0:64, :], rb_a[:])
                        nc.sync.dma_start(out=a2a_in[hh][dst], in_=cxc[:])
                nc.gpsimd.collective_compute(
                    "AllToAll", ALU.bypass,
                    replica_groups=[list(range(NC_))],
                    ins=[a2a_in[hh][:]], outs=[a2a_out[hh][:]],
                )

        att_stack.close()

        # wout prefetch (after attention SBUF is freed)
        wout_sb = p3w.tile([128, DT // 2, 2, ET, 128], FP8, tag="wout")
        nc.sync.dma_start(out=wout_sb, in_=wout_d[:])

        # ================= Phase 3a: out-proj + residual + cross stage ========
        x_stack = ExitStack()
        x_pool = x_stack.enter_context(tc.tile_pool(name="xp", bufs=1))
        x2 = x_pool.tile([128, ET, TOK], F32R, tag="x2")
        x3 = x_pool.tile([128, ET, TOK], F32R, tag="x3")
        h3 = x_pool.tile([128, ET, TOK], FP8, tag="h3")
        sq2 = x_pool.tile([128, ET, TOK], FP8, tag="sq2i")

        with (
            tc.tile_pool(name="p3c", bufs=1) as p3c,
            tc.tile_pool(name="p3op", bufs=1, space="PSUM") as p3op,
        ):
            cxt = [p3c.tile([128, 4, TOK], FP8, name=f"cxt{h}", tag=f"cxt{h}")
                   for h in range(2)]
            op_ps = [p3op.tile([128, TOK], F32, name=f"op{e}", tag=f"op{e}")
                     for e in range(ET)]
            for h2 in range(2):
                for kt in range(4):
                    nc.sync.dma_start(
                        out=cxt[h2][0:64, kt, :], in_=a2a_out[h2][2 * kt],
                    )
                    nc.sync.dma_start(
                        out=cxt[h2][64:128, kt, :], in_=a2a_out[h2][2 * kt + 1],
                    )
                for e in range(ET):
                    for j in range(2):
                        nc.tensor.matmul(
                            op_ps[e][:], wo_sb[h2][:, j, :, e, :],
                            cxt[h2][:, 2 * j:2 * j + 2, :],
                            start=(h2 == 0 and j == 0), stop=(h2 == 1 and j == 1),
                            perf_mode=DR,
                        )
                    if h2 == 1:
                        nc.vector.scalar_tensor_tensor(
                            out=x2[:, e, :], in0=op_ps[e][:], scalar=1.0 / SWA,
                            in1=xo_sb[:, e, :], op0=ALU.mult, op1=ALU.add,
                        )
                        if e % 2 == 0:
                            nc.scalar.activation(
                                sq2[:, e, :], x2[:, e, :], AF.Square,
                                scale=float(np.sqrt(SQ)),
                            )
                        else:
                            nc.vector.scalar_tensor_tensor(
                                out=sq2[:, e, :], in0=x2[:, e, :], scalar=SQ,
                                in1=x2[:, e, :], op0=ALU.mult, op1=ALU.mult,
                            )

        def rms_scale(src, tag, extra_scale, pool, pool_ps):
            """sum-of-squares over e-tiles of src -> broadcast scale tile."""
            sq8 = pool.tile([128, ET, TOK], FP8, tag=f"sq{tag}")
            for e in range(ET):
                if e % 2 == 0:
                    nc.scalar.activation(
                        sq8[:, e, :], src[:, e, :], AF.Square,
                        scale=float(np.sqrt(SQ)),
                    )
                else:
                    nc.vector.scalar_tensor_tensor(
                        out=sq8[:, e, :], in0=src[:, e, :], scalar=SQ,
                        in1=src[:, e, :], op0=ALU.mult, op1=ALU.mult,
                    )
            ss_ps = pool_ps.tile([1, TOK], F32, tag=f"ss{tag}")
            for i in range(ET // 2):
                nc.tensor.matmul(
                    ss_ps[:], ones8_sb[:, :, 0:1], sq8[:, 2 * i:2 * i + 2, :],
                    start=(i == 0), stop=(i == ET // 2 - 1), perf_mode=DR,
                )
            rinv = pool.tile([1, TOK], F32, tag=f"ri{tag}")
            nc.vector.reciprocal_approx_fast(out=rinv[:], in_=ss_ps[:])
            r = pool.tile([1, TOK], F32R, tag=f"r{tag}")
            nc.scalar.activation(
                r[:], rinv[:], AF.Sqrt, scale=float(E * SQ * extra_scale)
            )
            return r

        with (
            tc.tile_pool(name="p3n", bufs=2) as p3n,
            tc.tile_pool(name="p3nps", bufs=2, space="PSUM") as p3nps,
        ):
            ss2_ps = p3nps.tile([1, TOK], F32, tag="ss2")
            for i in range(ET // 2):
                nc.tensor.matmul(
                    ss2_ps[:], ones8_sb[:, :, 0:1], sq2[:, 2 * i:2 * i + 2, :],
                    start=(i == 0), stop=(i == ET // 2 - 1), perf_mode=DR,
                )
            rinv2 = p3n.tile([1, TOK], F32, tag="ri2")
            nc.vector.reciprocal_approx_fast(out=rinv2[:], in_=ss2_ps[:])
            r2 = p3n.tile([1, TOK], F32R, tag="r2")
            nc.scalar.activation(r2[:], rinv2[:], AF.Sqrt, scale=float(E * SQ))
            r2p = p3n.tile([1, TOK], F32R, tag="r2p")
            nc.vector.tensor_scalar(
                out=r2p[:], in0=r2[:], scalar1=1.0, scalar2=None, op0=ALU.add
            )
            rb2_ps = p3nps.tile([128, TOK], F32, tag="rb2ps")
            nc.tensor.matmul(rb2_ps[:], onesrow_sb[:], r2p[:], start=True, stop=True)
            rb2 = p3n.tile([128, TOK], F32R, tag="rb2")
            nc.scalar.activation(rb2[:], rb2_ps[:], AF.Copy)
            for e in range(ET):
                nc.vector.tensor_mul(x3[:, e, :], x2[:, e, :], rb2[:])

            r3 = rms_scale(x3, "3", SA * SA, p3n, p3nps)
            rb3_ps = p3nps.tile([128, TOK], F32, tag="rb2ps")
            nc.tensor.matmul(rb3_ps[:], onesrow_sb[:], r3[:], start=True, stop=True)
            rb3 = p3n.tile([128, TOK], F32R, tag="rb3")
            nc.scalar.activation(rb3[:], rb3_ps[:], AF.Copy)
            for e in range(ET):
                nc.vector.tensor_mul(h3[:, e, :], x3[:, e, :], rb3[:])

        # ================= Phase 3b: GLU FFN =================
        with (
            tc.tile_pool(name="g", bufs=1) as gpool,
            tc.tile_pool(name="f3w", bufs=4) as f3w,
            tc.tile_pool(name="f3o", bufs=3) as f3o,
        ):
            g = gpool.tile([128, DT, TOK], FP8, tag="g")
            uv_ps_pool = ExitStack()
            f3ps = uv_ps_pool.enter_context(
                tc.tile_pool(name="f3ps", bufs=4, space="PSUM")
            )
            for d in range(DT):
                wwv_c = f3w.tile([128, 2, 4, 2, 128], FP8, tag="wwvc")
                nc.sync.dma_start(out=wwv_c, in_=wwv_d[d])
                u_ps = f3ps.tile([128, TOK], F32, tag="mm")
                for i in range(ET // 2):
                    nc.tensor.matmul(
                        u_ps[:], wwv_c[:, 0, i, :, :], h3[:, 2 * i:2 * i + 2, :],
                        start=(i == 0), stop=(i == ET // 2 - 1), perf_mode=DR,
                    )
                v_ps = f3ps.tile([128, TOK], F32, tag="mm")
                for i in range(ET // 2):
                    nc.tensor.matmul(
                        v_ps[:], wwv_c[:, 1, i, :, :], h3[:, 2 * i:2 * i + 2, :],
                        start=(i == 0), stop=(i == ET // 2 - 1), perf_mode=DR,
                    )
                gl = f3o.tile([128, TOK], BF16, tag="gl")
                nc.scalar.activation(gl[:], u_ps[:], AF.Gelu, scale=1.0 / SWA)
                nc.vector.scalar_tensor_tensor(
                    out=g[:, d, :], in0=v_ps[:], scalar=SA / SWA, in1=gl[:],
                    op0=ALU.mult, op1=ALU.mult,
                )
            uv_ps_pool.close()

            # Wout + residual, e-outer, streamed output
            with tc.tile_pool(name="fps2", bufs=2, space="PSUM") as fps2:
                for e in range(ET):
                    f_ps = fps2.tile([128, TOK], F32, tag="fps")
                    for dp in range(DT // 2):
                        nc.tensor.matmul(
                            f_ps[:], wout_sb[:, dp, :, e, :],
                            g[:, 2 * dp:2 * dp + 2, :],
                            start=(dp == 0), stop=(dp == DT // 2 - 1),
                            perf_mode=DR,
                        )
                    y_sb = f3o.tile([128, TOK], BF16, tag="ysb")
                    nc.vector.scalar_tensor_tensor(
                        out=y_sb[:], in0=f_ps[:], scalar=1.0 / SWA,
                        in1=x3[:, e, :], op0=ALU.mult, op1=ALU.add,
                    )
                    nc.sync.dma_start(out=y[e], in_=y_sb[:])

        x_stack.close()
        p3w_stack.close()
        estack.close()

    nc.finalize()
    return nc


def make_in_maps(X, Wqkv, bqkv_in, Wo_sa, bo_sa, Ww, Wv, Wout):
    bf = ml_dtypes.bfloat16
    f8 = ml_dtypes.float8_e4m3
    Xf = np.ascontiguousarray(np.asarray(X, np.float32).reshape(T, E))
    XT = np.ascontiguousarray(Xf.T)  # [E, T]
    # chunk-major [8, 128, ET, 512]: contiguous 8KB per partition per chunk
    xt_bf = np.ascontiguousarray(
        XT.reshape(ET, 128, 8, T // 8).transpose(2, 1, 0, 3).astype(bf)
    )
    bo = np.asarray(bo_sa, np.float32)
    xt_own_all = XT + bo[:, None]  # [E, T] f32 with bias folded
    Wr = np.asarray(Wqkv, np.float32).reshape(E, H, 3, D)
    br = np.asarray(bqkv_in, np.float32).reshape(H, 3, D)
    pos = (np.arange(T, dtype=np.float32) % L)
    slopes_all = (2.0 ** (-np.linspace(1.0, 8.0, H))).astype(np.float32)
    ones_col = np.ones([128, 1], np.float32)
    ones8 = np.ones([128, 2, 16], f8)
    onesrow = np.ones([1, 128], np.float32)
    ones_t = np.ones([T], np.float32)

    def ext_hilo(h0):
        """[2, 4, T] bf16 hi/lo-split alibi rows for heads h0, h0+1."""
        ek = np.empty((2, 4, T), np.float32)
        eq = np.empty((2, 4, T), np.float32)
        for i, h in enumerate((h0, h0 + 1)):
            v = slopes_all[h] * pos
            hi = v.astype(bf).astype(np.float32)
            lo = v - hi
            ek[i] = np.stack([hi, lo, ones_t, ones_t])
            eq[i] = np.stack([ones_t, ones_t, -hi, -lo])
        return ek.astype(bf), eq.astype(bf)
    p_i = np.arange(128)[:, None, None]
    i_i = np.arange(2)[None, :, None]
    f_s = np.arange(QC)[None, None, :]
    mbs = np.where(f_s >= p_i + 128 * i_i, 0.0, -30000.0).astype(np.float32)
    f_d = np.arange(QC // 2)[None, None, :]
    mbd = np.where(f_d >= p_i + 128 * i_i, 0.0, -30000.0).astype(np.float32)
    ident = np.eye(128, dtype=np.float32)
    vones = np.ones([128, 64], f8)

    # out-proj fp8 x SW: wo_d[h2, p, ktp, j, e, m] =
    #   Wo[((2*(2ktp+j) + p//64)*2 + h2)*64 + p%64, e*128+m]
    Wo = np.asarray(Wo_sa, np.float32) * SW
    wo_t = np.empty((2, 128, 2, 2, ET, 128), np.float32)
    Wo5 = Wo.reshape(NC_, 2, 64, ET, 128)  # [src, hsel, d, e, m]
    for h2 in range(2):
        w = Wo5[:, h2]                      # [8 src, 64, e, m]
        # p = src_rel*64 + d; src = 2*(2ktp+j) + src_rel
        w = w.reshape(2, 2, 2, 64, ET, 128)  # [ktp, j, src_rel, d, e, m]
        wo_t[h2] = w.transpose(2, 3, 0, 1, 4, 5).reshape(128, 2, 2, ET, 128)
    wo_t8 = np.ascontiguousarray(wo_t.astype(f8))

    # wwv fp8 x SW: [DT, 128, 2(uv), 4(ep), 2(j), 128]
    W2 = np.stack([np.asarray(Ww, np.float32), np.asarray(Wv, np.float32)], 0) * SW
    # [uv, e, p, d, m] -> [d, p, uv, ep, j, m] with e = 2*ep + j
    wwv = W2.reshape(2, ET // 2, 2, 128, DT, 128).transpose(4, 3, 0, 1, 2, 5)
    wwv_t8 = np.ascontiguousarray(wwv.astype(f8))

    # wout fp8 x SW: [128, DT//2, 2, ET, 128]; k-row = (2*dp + j)*128 + p
    Wo2 = np.asarray(Wout, np.float32) * SW
    wout = Wo2.reshape(DT // 2, 2, 128, ET, 128).transpose(2, 0, 1, 3, 4)
    wout_t8 = np.ascontiguousarray(wout.astype(f8))

    in_maps = []
    for c in range(NC_):
        h0 = 2 * c
        extk, extq = ext_hilo(h0)
        wq_p = np.concatenate([Wr[:, h0, 0, :], Wr[:, h0 + 1, 0, :]], axis=1) * 0.125
        wk_p = np.concatenate([Wr[:, h0, 1, :], Wr[:, h0 + 1, 1, :]], axis=1)
        wv_p = np.concatenate([Wr[:, h0, 2, :], Wr[:, h0 + 1, 2, :]], axis=1)
        wqkv_t = np.stack([wq_p, wk_p, wv_p], axis=1)      # [E, 3, 128]
        wqkv_t = np.ascontiguousarray(
            wqkv_t.reshape(ET, 128, 3, 128).transpose(1, 0, 2, 3).astype(bf)
        )  # [128, ET, 3, 128]
        bq_p = np.concatenate([br[h0, 0], br[h0 + 1, 0]]) * 0.125
        bk_p = np.concatenate([br[h0, 1], br[h0 + 1, 1]])
        bv_p = np.concatenate([br[h0, 2], br[h0 + 1, 2]])
        bq3 = np.stack([bq_p, bk_p, bv_p], axis=1).astype(np.float32)  # [128,3]
        xt_own = np.ascontiguousarray(
            xt_own_all[:, TOK * c:TOK * (c + 1)]
            .reshape(ET, 128, TOK).transpose(1, 0, 2).astype(bf)
        )
        in_maps.append({
            "xt_bf": xt_bf,
            "xt_own": xt_own,
            "wqkv_d": wqkv_t,
            "bqkv": bq3,
            "wo_d": wo_t8,
            "wwv_d": wwv_t8,
            "wout_d": wout_t8,
            "onesd": ones_col,
            "ones8d": ones8,
            "onesrowd": onesrow,
            "extkd": extk,
            "extqd": extq,
            "mbsd": mbs,
            "mbdd": mbd,
            "identd": ident,
            "vonesd": vones,
        })
    return in_maps


def kernel(**inputs) -> np.ndarray:
    out, _ = run(inputs, trace=False)
    return out


def run(inputs, trace=False):
    if "nc" not in _CACHE:
        _CACHE["nc"] = build_nc()
    nc = _CACHE["nc"]
    in_maps = make_in_maps(
        inputs["X"], inputs["Wqkv"], inputs["bqkv"], inputs["Wo_sa"],
        inputs["bo_sa"], inputs["Ww"], inputs["Wv"], inputs["Wout"],
    )
    if trace:
        install_ntff_hook()
    res = run_bass_kernel_spmd(nc, in_maps, list(range(NC_)), trace=trace)
    out = np.concatenate(
        [np.asarray(r["y"]).astype(np.float32).reshape(E, TOK)
         for r in res.results], axis=1
    )  # [E, T]
    return np.ascontiguousarray(out.T).reshape(B, L, E).astype(np.float32), res
